# revision 8
# baseline (speedup 1.0000x reference)
"""MoE router (top-2 gating) Trainium2 Bass kernel, SPMD over 8 NeuronCores.

Problem: x [4, 4096, 2048] f32, gate_w [64, 2048] f32.
  logits = x @ gate_w.T          -> [4, 4096, 64]
  scores, indices = top_k(logits, 2)
  weights = softmax(scores)      -> ([4, 4096, 2] f32, [4, 4096, 2] i32)

Sharding: data-parallel over tokens; each of the 8 cores gets 2048 tokens.

DMA layout (measured on HW):
  - chunk 0 (tokens 0:1024, 8 MiB) ships as host-packed SBUF images: each
    sub-DMA is a plain [128 x 4096] f32 block copy whose per-partition row
    is 16 KiB contiguous -> ~425 GB/s. Contiguous sources this big spread
    across all 16 SDMA engines (128 KiB granules).
  - chunks 1/2 (tokens 1024:2048) ship from a [D, 1024] matrix as
    [p, dt, t]-strided sub-DMAs (2 KiB elements). Strided multi-element
    descriptors are engine-assigned by partition port, so even the tiny
    1-d-tile tail subs keep all 16 engines busy (a small contiguous source
    would collapse onto 1-2 engines and trickle at ~25 GB/s).

Per-core device pipeline:
  - ALL x sub-DMA triggers are issued up front on the sync queue so nothing
    can stall the input stream
  - token chunks [1024, 512, 512]: col-packed fp32 matmul pairs per d-tile
    accumulate logits.T in one PSUM bank per chunk (A-group rows 0:64,
    B-group rows 64:128); the final chunk ends in 1-d-tile sub-DMAs so
    almost no matmul work remains after the last byte lands
  - per-chunk epilogue: PSUM->SBUF quadrant copies (DVE+ACT), PE
    back-transpose into PSUM, hardware top-8 (max8, then softmax inputs,
    then max_index) reading PSUM directly, top-2 softmax as two sigmoid
    ACT ops
  - one merged output DMA at the end (64 KiB per core)
  - host unpermutes the [128, 16, 2] per-core buffers to token order
"""
import sys

if "/opt/trn_rl_repo" not in sys.path:
    sys.path.insert(0, "/opt/trn_rl_repo")

import numpy as np

B, T, D, E, K = 4, 4096, 2048, 64, 2
N_CORES = 8
P = 128
NDT = D // P                      # 16 d-tiles
TOK_PER_CORE = B * T // N_CORES   # 2048
NSEG = TOK_PER_CORE // P          # 16 output segments of 128 tokens

_compiled = None

# (t0, ntok) token chunks; epilogue granularity = chunk
CHUNKS = [(0, 1024), (1024, 512), (1536, 256), (1792, 256)]
# per-chunk x sub-DMA splits (d0, nd); final chunk tails off in 1-d-tile
# subs so the PE never has a backlog when the last byte lands
SPLITS = {
    0:    [(0, 4), (4, 4), (8, 4), (12, 4)],
    1024: [(0, 4), (4, 4), (8, 4), (12, 4)],
    1536: [(0, 8), (8, 8)],
    1792: [(0, 8), (8, 4), (12, 2), (14, 1), (15, 1)],
}
C0_W = 4 * 1024                   # packed columns per chunk-0 sub


def _build():
    import concourse.bacc as bacc
    import concourse.tile as tile
    from concourse import mybir
    from concourse.masks import make_identity

    nc = bacc.Bacc("TRN2", target_bir_lowering=False, debug=False,
                   num_devices=N_CORES)

    # chunk 0: host-packed [128, 4*4096] image (sub-major)
    xpk_in = nc.dram_tensor("xpk", [P, 4 * C0_W], mybir.dt.float32,
                            kind="ExternalInput")
    # chunks 1/2: [D, 1024] slice of the transposed shard (tokens 1024:2048)
    xT_in = nc.dram_tensor("xT2", [D, 1024], mybir.dt.float32,
                           kind="ExternalInput")
    gw_in = nc.dram_tensor("gwl", [P, NDT * E], mybir.dt.float32,
                           kind="ExternalInput")
    # single merged output: [:, 0:NSEG*K] = weight bits (f32), rest = indices
    o_out = nc.dram_tensor("o", [P, NSEG * K * 2], mybir.dt.uint32,
                           kind="ExternalOutput")

    fp32 = mybir.dt.float32
    Sig = mybir.ActivationFunctionType.Sigmoid

    with tile.TileContext(nc) as tc:
        with (
            tc.tile_pool(name="xpool", bufs=1) as xpool,
            tc.tile_pool(name="cpool", bufs=1) as cpool,
            tc.tile_pool(name="epool", bufs=1) as epool,
            tc.tile_pool(name="opool", bufs=1) as opool,
            tc.tile_pool(name="pacc", bufs=1, space="PSUM") as pacc,
            tc.tile_pool(name="plg", bufs=2, space="PSUM") as plg,
        ):
            # constants / one-time loads
            gw_sb = cpool.tile([P, NDT * E], fp32)
            nc.sync.dma_start(gw_sb[:], gw_in.ap())
            ident = cpool.tile([P, P], fp32)
            make_identity(nc, ident[:])
            # warm the ACT sigmoid table early (overlaps the input stream)
            scratch = cpool.tile([P, 1], fp32)
            nc.gpsimd.memset(scratch[:], 0.0)
            nc.scalar.activation(scratch[:], scratch[:], Sig)

            # merged output accumulator: weights bits then indices
            acc_all = opool.tile([P, NSEG * K * 2], mybir.dt.uint32)
            wv = acc_all[:, 0:NSEG * K].bitcast(fp32).rearrange(
                "p (s k) -> p s k", k=K)

            xT_v = xT_in.ap().rearrange("(dt p) t -> p dt t", p=P)

            # ---- phase 1: issue EVERY input sub-DMA before anything that
            # could block the sync queue ----
            subs = {}
            for si, (d0, nd) in enumerate(SPLITS[0]):
                xt = xpool.tile([P, C0_W], fp32, tag=f"x0_{d0}",
                                name=f"xt_0_{d0}")
                nc.sync.dma_start(xt[:],
                                  xpk_in.ap()[:, si * C0_W:(si + 1) * C0_W])
                subs[(0, d0, nd)] = xt
            for (t0, ntok) in CHUNKS[1:]:
                for (d0, nd) in SPLITS[t0]:
                    xt = xpool.tile([P, nd * ntok], fp32,
                                    tag=f"x{t0}_{d0}", name=f"xt_{t0}_{d0}")
                    nc.sync.dma_start(
                        xt[:].rearrange("p (dt t) -> p dt t", dt=nd),
                        xT_v[:, d0:d0 + nd, t0 - 1024:t0 - 1024 + ntok],
                    )
                    subs[(t0, d0, nd)] = xt

            def find_src(t0, ntok, dt):
                for (ct0, d0, nd), xt in subs.items():
                    if ct0 == t0 and d0 <= dt < d0 + nd:
                        return xt, (dt - d0) * ntok
                raise AssertionError

            # ---- phase 2: per-chunk matmuls + epilogue ----
            for ci, (t0, ntok) in enumerate(CHUNKS):
                half = ntok // 2
                nblk = ntok // P
                s0 = t0 // P

                # col-packed fp32 matmul pair per d-tile: group A (tokens
                # [0, half)) -> PSUM rows 0:64, group B -> rows 64:128
                acc = pacc.tile([P, half], fp32, tag=f"acc{ci}",
                                name=f"acc{ci}")
                for dt in range(NDT):
                    src, base = find_src(t0, ntok, dt)
                    gsl = gw_sb[:, dt * E:(dt + 1) * E]
                    mmargs = dict(start=(dt == 0), stop=(dt == NDT - 1))
                    nc.tensor.matmul(acc[0:64, :], gsl,
                                     src[:, base:base + half],
                                     tile_position=(0, 0), **mmargs)
                    nc.tensor.matmul(acc[64:128, :], gsl,
                                     src[:, base + half:base + ntok],
                                     tile_position=(0, 64), **mmargs)

                # quadrant copies into token-aligned [128, ntok] layout
                # (both on DVE: ACT pays ~500ns sem-receive latency, and the
                # transposes can't start until the B quadrant lands)
                lt = epool.tile([P, ntok], fp32, tag=f"lt{ci}",
                                name=f"lt{ci}")
                nc.vector.tensor_copy(lt[0:64, 0:half], acc[0:64, :])
                nc.vector.tensor_copy(lt[64:128, half:ntok], acc[64:128, :])

                # back-transpose per 128-token block into PSUM; top-8 reads
                # PSUM directly (no second PSUM->SBUF copy)
                mx = opool.tile([P, nblk * 8], fp32, tag=f"mx{ci}",
                                name=f"mx{ci}")
                mi = opool.tile([P, nblk * 8], mybir.dt.uint32,
                                tag=f"mi{ci}", name=f"mi{ci}")
                mi3 = mi[:].rearrange("p (s k) -> p s k", k=8)
                mx3 = mx[:].rearrange("p (s k) -> p s k", k=8)

                lgs = []
                for b0 in range(0, nblk, 4):
                    nb = min(4, nblk - b0)
                    lg = plg.tile([P, 512], fp32, tag="lg",
                                  name=f"lg{ci}_{b0}")
                    lgs.append(lg)
                    for j in range(b0, b0 + nb):
                        nc.tensor.transpose(
                            lg[:, (j - b0) * P:(j - b0 + 1) * P],
                            lt[:, j * P:(j + 1) * P], ident[:],
                        )
                    # max8 first: the softmax chain depends only on these
                    for j in range(b0, b0 + nb):
                        cb = 0 if j < nblk // 2 else 64
                        seg = lg[:, (j - b0) * P + cb:(j - b0) * P + cb + 64]
                        nc.vector.max(out=mx[:, j * 8:(j + 1) * 8], in_=seg)

                # top-2 softmax == sigmoid of the score gap (both weights);
                # runs on ACT while the DVE does the index finds below
                delta = epool.tile([P, nblk], fp32, tag=f"dl{ci}",
                                   name=f"dl{ci}")
                nc.vector.tensor_tensor(delta[:], mx3[:, :, 1], mx3[:, :, 0],
                                        op=mybir.AluOpType.subtract)
                nc.scalar.activation(wv[:, s0:s0 + nblk, 1], delta[:], Sig)
                nc.scalar.activation(wv[:, s0:s0 + nblk, 0], delta[:], Sig,
                                     scale=-1.0)

                for j in range(nblk):
                    cb = 0 if j < nblk // 2 else 64
                    lg = lgs[j // 4]
                    seg = lg[:, (j % 4) * P + cb:(j % 4) * P + cb + 64]
                    nc.vector.max_index(
                        mi[:, j * 8:(j + 1) * 8],
                        mx[:, j * 8:(j + 1) * 8], seg,
                    )
                nc.vector.tensor_copy(
                    acc_all[:, NSEG * K + s0 * K: NSEG * K + (s0 + nblk) * K]
                    .rearrange("p (s k) -> p s k", k=K),
                    mi3[:, :, 0:K])

            # ---- phase 3: one merged output DMA ----
            nc.sync.dma_start(o_out.ap(), acc_all[:])

    nc.compile()
    return nc


def _get_compiled():
    global _compiled
    if _compiled is None:
        _compiled = _build()
    return _compiled


def kernel(x, gate_w):
    from concourse.bass_utils import run_bass_kernel_spmd

    x = np.ascontiguousarray(np.asarray(x, dtype=np.float32))
    gate_w = np.ascontiguousarray(np.asarray(gate_w, dtype=np.float32))
    assert x.shape == (B, T, D) and gate_w.shape == (E, D)

    nc = _get_compiled()

    x_flat = x.reshape(B * T, D)
    # gate_w.T laid out [128, 16*64]: (p, dt*64+e) = gate_w[e, dt*128+p]
    gwl = np.ascontiguousarray(
        gate_w.T.reshape(NDT, P, E).transpose(1, 0, 2).reshape(P, NDT * E)
    )

    from concurrent.futures import ThreadPoolExecutor

    def shard(c):
        sl = x_flat[c * TOK_PER_CORE:(c + 1) * TOK_PER_CORE]
        xT = np.ascontiguousarray(sl.T)            # [D, TOK_PER_CORE]
        xs = xT.reshape(NDT, P, TOK_PER_CORE)      # [dt, p, t]
        # chunk 0 packed: per sub [p, nd*1024] image, subs concatenated
        blocks = []
        for (d0, nd) in SPLITS[0]:
            blk = xs[d0:d0 + nd, :, 0:1024]        # [nd, p, t]
            blocks.append(np.ascontiguousarray(
                blk.transpose(1, 0, 2)).reshape(P, nd * 1024))
        xpk = np.concatenate(blocks, axis=1)       # [P, 4*4096]
        xT2 = np.ascontiguousarray(xT[:, 1024:2048])
        return xpk, xT2

    with ThreadPoolExecutor(max_workers=N_CORES) as ex:
        shards = list(ex.map(shard, range(N_CORES)))

    in_maps = [{"xpk": shards[c][0], "xT2": shards[c][1], "gwl": gwl}
               for c in range(N_CORES)]
    res = run_bass_kernel_spmd(nc, in_maps, list(range(N_CORES)))

    # device buffer is [P, 2*NSEG*K] u32: first half f32 weight bits,
    # second half indices; token = s*128 + p
    def unperm(buf):
        return buf.reshape(P, NSEG, K).transpose(1, 0, 2).reshape(
            TOK_PER_CORE, K)

    ws, idxs = [], []
    for c in range(N_CORES):
        o = res.results[c]["o"]
        ws.append(unperm(o[:, :NSEG * K].view(np.float32)))
        idxs.append(unperm(o[:, NSEG * K:]))
    weights = np.concatenate(ws, axis=0).reshape(B, T, K).astype(np.float32)
    indices = np.concatenate(idxs, axis=0).reshape(B, T, K).astype(np.int32)
    return weights, indices


# revision 9
# speedup vs baseline: 1.0149x; 1.0149x over previous
"""MoE router (top-2 gating) Trainium2 Bass kernel, SPMD over 8 NeuronCores.

Problem: x [4, 4096, 2048] f32, gate_w [64, 2048] f32.
  logits = x @ gate_w.T          -> [4, 4096, 64]
  scores, indices = top_k(logits, 2)
  weights = softmax(scores)      -> ([4, 4096, 2] f32, [4, 4096, 2] i32)

Sharding: data-parallel over tokens; each of the 8 cores gets 2048 tokens.

DMA layout (measured on HW):
  - chunks 0/1 (tokens 0:1536, 12.5 MiB) ship as host-packed SBUF images:
    each sub-DMA is a plain [128 x 4096] f32 block copy whose per-partition
    row is 16 KiB contiguous -> ~425 GB/s. Contiguous sources this big
    spread across all 16 SDMA engines (128 KiB granules).
  - chunk 2 (tokens 1536:2048) ships from a [D, 512] matrix as
    [p, dt, t]-strided sub-DMAs (2 KiB elements, ~387 GB/s). Strided
    multi-element descriptors are engine-assigned by partition port, so
    even the 1-d-tile tail subs keep all 16 engines busy; a small
    contiguous source would collapse onto 1-2 engines (128 KiB granules)
    and trickle at ~25 GB/s. 1 KiB elements are also bad (~300 GB/s), so
    the last chunk stays at 512 tokens.

Per-core device pipeline:
  - ALL x sub-DMA triggers are issued up front on the sync queue so
    nothing can stall the input stream
  - token chunks [1024, 512, 512]: col-packed fp32 matmul pairs per d-tile
    accumulate logits.T in one PSUM bank per chunk (A-group rows 0:64,
    B-group rows 64:128); the final chunk ends in 1-d-tile sub-DMAs so
    almost no matmul work remains after the last byte lands
  - per-chunk epilogue: PSUM->SBUF quadrant copies (both DVE; ACT pays
    ~500ns sem-receive latency), PE back-transpose into PSUM in 2-block
    batches so max8 starts after 2 transposes instead of 4, hardware top-8
    (max8, then the sigmoid softmax inputs, then max_index) reading PSUM
    directly, top-2 softmax as two sigmoid ACT ops overlapping the finds
  - one merged output DMA at the end (64 KiB per core)
  - host unpermutes the [128, 16, 2] per-core buffers to token order
"""
import sys

if "/opt/trn_rl_repo" not in sys.path:
    sys.path.insert(0, "/opt/trn_rl_repo")

import numpy as np

B, T, D, E, K = 4, 4096, 2048, 64, 2
N_CORES = 8
P = 128
NDT = D // P                      # 16 d-tiles
TOK_PER_CORE = B * T // N_CORES   # 2048
NSEG = TOK_PER_CORE // P          # 16 output segments of 128 tokens

_compiled = None

# (t0, ntok) token chunks; epilogue granularity = chunk
CHUNKS = [(0, 1024), (1024, 512), (1536, 512)]
# packed sub-DMAs (chunks 0/1): (t0, d0, nd), each nd*ntok == 4096 cols
PACKED_SUBS = [(0, 0, 4), (0, 4, 4), (0, 8, 4), (0, 12, 4),
               (1024, 0, 8), (1024, 8, 8)]
# strided sub-DMAs (chunk 2): (d0, nd); tails off in 1-d-tile subs so the
# PE never has a backlog when the last byte lands
C2_SPLITS = [(0, 4), (4, 4), (8, 2), (10, 2), (12, 1), (13, 1), (14, 1),
             (15, 1)]
PK_W = 4096                       # packed columns per sub


def _build():
    import concourse.bacc as bacc
    import concourse.tile as tile
    from concourse import mybir
    from concourse.masks import make_identity

    nc = bacc.Bacc("TRN2", target_bir_lowering=False, debug=False,
                   num_devices=N_CORES)

    # chunks 0/1: host-packed [128, 6*4096] image (sub-major)
    xpk_in = nc.dram_tensor("xpk", [P, len(PACKED_SUBS) * PK_W],
                            mybir.dt.float32, kind="ExternalInput")
    # chunk 2: [D, 512] slice of the transposed shard (tokens 1536:2048)
    xT_in = nc.dram_tensor("xT3", [D, 512], mybir.dt.float32,
                           kind="ExternalInput")
    gw_in = nc.dram_tensor("gwl", [P, NDT * E], mybir.dt.float32,
                           kind="ExternalInput")
    # single merged output: [:, 0:NSEG*K] = weight bits (f32), rest = indices
    o_out = nc.dram_tensor("o", [P, NSEG * K * 2], mybir.dt.uint32,
                           kind="ExternalOutput")

    fp32 = mybir.dt.float32
    Sig = mybir.ActivationFunctionType.Sigmoid

    with tile.TileContext(nc) as tc:
        with (
            tc.tile_pool(name="xpool", bufs=1) as xpool,
            tc.tile_pool(name="cpool", bufs=1) as cpool,
            tc.tile_pool(name="epool", bufs=1) as epool,
            tc.tile_pool(name="opool", bufs=1) as opool,
            tc.tile_pool(name="pacc", bufs=1, space="PSUM") as pacc,
            tc.tile_pool(name="plg", bufs=4, space="PSUM") as plg,
        ):
            # constants / one-time loads
            gw_sb = cpool.tile([P, NDT * E], fp32)
            nc.sync.dma_start(gw_sb[:], gw_in.ap())
            ident = cpool.tile([P, P], fp32)
            make_identity(nc, ident[:])
            # warm the ACT sigmoid table early (overlaps the input stream)
            scratch = cpool.tile([P, 1], fp32)
            nc.gpsimd.memset(scratch[:], 0.0)
            nc.scalar.activation(scratch[:], scratch[:], Sig)

            # merged output accumulator: weights bits then indices
            acc_all = opool.tile([P, NSEG * K * 2], mybir.dt.uint32)
            wv = acc_all[:, 0:NSEG * K].bitcast(fp32).rearrange(
                "p (s k) -> p s k", k=K)

            xT_v = xT_in.ap().rearrange("(dt p) t -> p dt t", p=P)

            # ---- phase 1: issue EVERY input sub-DMA before anything that
            # could block the sync queue ----
            subs = {}
            for si, (t0, d0, nd) in enumerate(PACKED_SUBS):
                xt = xpool.tile([P, PK_W], fp32, tag=f"x{t0}_{d0}",
                                name=f"xt_{t0}_{d0}")
                nc.sync.dma_start(xt[:],
                                  xpk_in.ap()[:, si * PK_W:(si + 1) * PK_W])
                subs[(t0, d0, nd)] = xt
            for (d0, nd) in C2_SPLITS:
                xt = xpool.tile([P, nd * 512], fp32,
                                tag=f"x1536_{d0}", name=f"xt_1536_{d0}")
                nc.sync.dma_start(
                    xt[:].rearrange("p (dt t) -> p dt t", dt=nd),
                    xT_v[:, d0:d0 + nd, :],
                )
                subs[(1536, d0, nd)] = xt

            def find_src(t0, ntok, dt):
                for (ct0, d0, nd), xt in subs.items():
                    if ct0 == t0 and d0 <= dt < d0 + nd:
                        return xt, (dt - d0) * ntok
                raise AssertionError

            # ---- phase 2: per-chunk matmuls + epilogue ----
            for ci, (t0, ntok) in enumerate(CHUNKS):
                half = ntok // 2
                nblk = ntok // P
                s0 = t0 // P

                # col-packed fp32 matmul pair per d-tile: group A (tokens
                # [0, half)) -> PSUM rows 0:64, group B -> rows 64:128
                acc = pacc.tile([P, half], fp32, tag=f"acc{ci}",
                                name=f"acc{ci}")
                for dt in range(NDT):
                    src, base = find_src(t0, ntok, dt)
                    gsl = gw_sb[:, dt * E:(dt + 1) * E]
                    mmargs = dict(start=(dt == 0), stop=(dt == NDT - 1))
                    nc.tensor.matmul(acc[0:64, :], gsl,
                                     src[:, base:base + half],
                                     tile_position=(0, 0), **mmargs)
                    nc.tensor.matmul(acc[64:128, :], gsl,
                                     src[:, base + half:base + ntok],
                                     tile_position=(0, 64), **mmargs)

                # quadrant copies into token-aligned [128, ntok] layout
                lt = epool.tile([P, ntok], fp32, tag=f"lt{ci}",
                                name=f"lt{ci}")
                nc.vector.tensor_copy(lt[0:64, 0:half], acc[0:64, :])
                nc.vector.tensor_copy(lt[64:128, half:ntok], acc[64:128, :])

                # back-transpose per 128-token block into PSUM in 2-block
                # batches; top-8 reads PSUM directly
                mx = opool.tile([P, nblk * 8], fp32, tag=f"mx{ci}",
                                name=f"mx{ci}")
                mi = opool.tile([P, nblk * 8], mybir.dt.uint32,
                                tag=f"mi{ci}", name=f"mi{ci}")
                mi3 = mi[:].rearrange("p (s k) -> p s k", k=8)
                mx3 = mx[:].rearrange("p (s k) -> p s k", k=8)

                lgs = {}
                for b0 in range(0, nblk, 2):
                    lg = plg.tile([P, 256], fp32, tag="lg",
                                  name=f"lg{ci}_{b0}")
                    for j in (b0, b0 + 1):
                        lgs[j] = (lg, (j - b0) * P)
                        nc.tensor.transpose(
                            lg[:, (j - b0) * P:(j - b0 + 1) * P],
                            lt[:, j * P:(j + 1) * P], ident[:],
                        )
                    # max8 first: the softmax chain depends only on these
                    for j in (b0, b0 + 1):
                        cb = 0 if j < nblk // 2 else 64
                        lgt, lo = lgs[j]
                        nc.vector.max(out=mx[:, j * 8:(j + 1) * 8],
                                      in_=lgt[:, lo + cb:lo + cb + 64])

                # top-2 softmax == sigmoid of the score gap (both weights);
                # runs on ACT while the DVE does the index finds below
                delta = epool.tile([P, nblk], fp32, tag=f"dl{ci}",
                                   name=f"dl{ci}")
                nc.vector.tensor_tensor(delta[:], mx3[:, :, 1], mx3[:, :, 0],
                                        op=mybir.AluOpType.subtract)
                nc.scalar.activation(wv[:, s0:s0 + nblk, 1], delta[:], Sig)
                nc.scalar.activation(wv[:, s0:s0 + nblk, 0], delta[:], Sig,
                                     scale=-1.0)

                for j in range(nblk):
                    cb = 0 if j < nblk // 2 else 64
                    lgt, lo = lgs[j]
                    nc.vector.max_index(
                        mi[:, j * 8:(j + 1) * 8],
                        mx[:, j * 8:(j + 1) * 8],
                        lgt[:, lo + cb:lo + cb + 64],
                    )
                nc.vector.tensor_copy(
                    acc_all[:, NSEG * K + s0 * K: NSEG * K + (s0 + nblk) * K]
                    .rearrange("p (s k) -> p s k", k=K),
                    mi3[:, :, 0:K])

            # ---- phase 3: one merged output DMA ----
            nc.sync.dma_start(o_out.ap(), acc_all[:])

    nc.compile()
    return nc


def _get_compiled():
    global _compiled
    if _compiled is None:
        _compiled = _build()
    return _compiled


def kernel(x, gate_w):
    from concourse.bass_utils import run_bass_kernel_spmd

    x = np.ascontiguousarray(np.asarray(x, dtype=np.float32))
    gate_w = np.ascontiguousarray(np.asarray(gate_w, dtype=np.float32))
    assert x.shape == (B, T, D) and gate_w.shape == (E, D)

    nc = _get_compiled()

    x_flat = x.reshape(B * T, D)
    # gate_w.T laid out [128, 16*64]: (p, dt*64+e) = gate_w[e, dt*128+p]
    gwl = np.ascontiguousarray(
        gate_w.T.reshape(NDT, P, E).transpose(1, 0, 2).reshape(P, NDT * E)
    )

    from concurrent.futures import ThreadPoolExecutor

    def shard(c):
        sl = x_flat[c * TOK_PER_CORE:(c + 1) * TOK_PER_CORE]
        xT = np.ascontiguousarray(sl.T)            # [D, TOK_PER_CORE]
        xs = xT.reshape(NDT, P, TOK_PER_CORE)      # [dt, p, t]
        blocks = []
        for (t0, d0, nd) in PACKED_SUBS:
            ntok = dict(CHUNKS)[t0]
            blk = xs[d0:d0 + nd, :, t0:t0 + ntok]  # [nd, p, t]
            blocks.append(np.ascontiguousarray(
                blk.transpose(1, 0, 2)).reshape(P, nd * ntok))
        xpk = np.concatenate(blocks, axis=1)       # [P, 6*4096]
        xT3 = np.ascontiguousarray(xT[:, 1536:2048])
        return xpk, xT3

    with ThreadPoolExecutor(max_workers=N_CORES) as ex:
        shards = list(ex.map(shard, range(N_CORES)))

    in_maps = [{"xpk": shards[c][0], "xT3": shards[c][1], "gwl": gwl}
               for c in range(N_CORES)]
    res = run_bass_kernel_spmd(nc, in_maps, list(range(N_CORES)))

    # device buffer is [P, 2*NSEG*K] u32: first half f32 weight bits,
    # second half indices; token = s*128 + p
    def unperm(buf):
        return buf.reshape(P, NSEG, K).transpose(1, 0, 2).reshape(
            TOK_PER_CORE, K)

    ws, idxs = [], []
    for c in range(N_CORES):
        o = res.results[c]["o"]
        ws.append(unperm(o[:, :NSEG * K].view(np.float32)))
        idxs.append(unperm(o[:, NSEG * K:]))
    weights = np.concatenate(ws, axis=0).reshape(B, T, K).astype(np.float32)
    indices = np.concatenate(idxs, axis=0).reshape(B, T, K).astype(np.int32)
    return weights, indices


# revision 10
# speedup vs baseline: 1.1205x; 1.1040x over previous
"""MoE router (top-2 gating) Trainium2 Bass kernel, SPMD over 8 NeuronCores.

Problem: x [4, 4096, 2048] f32, gate_w [64, 2048] f32.
  logits = x @ gate_w.T          -> [4, 4096, 64]
  scores, indices = top_k(logits, 2)
  weights = softmax(scores)      -> ([4, 4096, 2] f32, [4, 4096, 2] i32)

Sharding: data-parallel over tokens; each of the 8 cores gets 2048 tokens.

DMA layout (measured on HW):
  - chunks 0/1 (tokens 0:1536, 12.5 MiB) ship as host-packed SBUF images:
    each sub-DMA is a plain [128 x 4096] f32 block copy whose per-partition
    row is 16 KiB contiguous -> ~425 GB/s. Contiguous sources this big
    spread across all 16 SDMA engines (128 KiB granules).
  - chunk 2 (tokens 1536:2048) ships from a [D, 512] matrix as
    [p, dt, t]-strided sub-DMAs (2 KiB elements, ~387 GB/s). Strided
    multi-element descriptors are engine-assigned by partition port, so
    even the 1-d-tile tail subs keep all 16 engines busy; a small
    contiguous source would collapse onto 1-2 engines (128 KiB granules)
    and trickle at ~25 GB/s. 1 KiB elements are also bad (~300 GB/s), so
    the last chunk stays at 512 tokens.

Per-core device pipeline:
  - ALL x sub-DMA triggers are issued up front on the sync queue so
    nothing can stall the input stream
  - token chunks [1024, 512, 512]: col-packed fp32 matmul pairs per d-tile
    accumulate logits.T in one PSUM bank per chunk (A-group rows 0:64,
    B-group rows 64:128); the final chunk ends in 1-d-tile sub-DMAs so
    almost no matmul work remains after the last byte lands
  - per-chunk epilogue: PSUM->SBUF quadrant copies (both DVE; ACT pays
    ~500ns sem-receive latency), PE back-transpose into PSUM in 2-block
    batches so max8 starts after 2 transposes instead of 4, hardware top-8
    (max8, then the sigmoid softmax inputs, then max_index) reading PSUM
    directly, top-2 softmax as two sigmoid ACT ops overlapping the finds
  - one merged output DMA at the end (64 KiB per core)
  - host unpermutes the [128, 16, 2] per-core buffers to token order
"""
import sys

if "/opt/trn_rl_repo" not in sys.path:
    sys.path.insert(0, "/opt/trn_rl_repo")

import numpy as np

B, T, D, E, K = 4, 4096, 2048, 64, 2
N_CORES = 8
P = 128
NDT = D // P                      # 16 d-tiles
TOK_PER_CORE = B * T // N_CORES   # 2048
NSEG = TOK_PER_CORE // P          # 16 output segments of 128 tokens

_compiled = None

# (t0, ntok) token chunks; epilogue granularity = chunk
CHUNKS = [(0, 1024), (1024, 512), (1536, 512)]
# packed sub-DMAs (chunks 0/1): (t0, d0, nd), each nd*ntok == 4096 cols
PACKED_SUBS = [(0, 0, 4), (0, 4, 4), (0, 8, 4), (0, 12, 4),
               (1024, 0, 8), (1024, 8, 8)]
# strided sub-DMAs (chunk 2): (d0, nd); tails off in 1-d-tile subs so the
# PE never has a backlog when the last byte lands
C2_SPLITS = [(0, 4), (4, 4), (8, 2), (10, 2), (12, 1), (13, 1), (14, 1),
             (15, 1)]
PK_W = 4096                       # packed columns per sub


def _build():
    import concourse.bacc as bacc
    import concourse.tile as tile
    from concourse import mybir
    from concourse.masks import make_identity

    nc = bacc.Bacc("TRN2", target_bir_lowering=False, debug=False,
                   num_devices=N_CORES)

    # chunks 0/1: host-packed [128, 6*4096] image (sub-major)
    xpk_in = nc.dram_tensor("xpk", [P, len(PACKED_SUBS) * PK_W],
                            mybir.dt.float32, kind="ExternalInput")
    # chunk 2: [D, 1024] slice of the transposed shard (tokens
    # 1024:2048). Chunk 2 only reads columns 512:1024 -- the unused first
    # half of every 4 KiB row leaves address gaps between the 2 KiB
    # elements, which forces the HWDGE into balanced per-partition engine
    # assignment (a fully-contiguous source collapses onto few engines).
    xT_in = nc.dram_tensor("xT2", [D, 1024], mybir.dt.float32,
                           kind="ExternalInput")
    gw_in = nc.dram_tensor("gwl", [P, NDT * E], mybir.dt.float32,
                           kind="ExternalInput")
    # single merged output: [:, 0:NSEG*K] = weight bits (f32), rest = indices
    o_out = nc.dram_tensor("o", [P, NSEG * K * 2], mybir.dt.uint32,
                           kind="ExternalOutput")

    fp32 = mybir.dt.float32
    Sig = mybir.ActivationFunctionType.Sigmoid

    with tile.TileContext(nc) as tc:
        with (
            tc.tile_pool(name="xpool", bufs=1) as xpool,
            tc.tile_pool(name="cpool", bufs=1) as cpool,
            tc.tile_pool(name="epool", bufs=1) as epool,
            tc.tile_pool(name="opool", bufs=1) as opool,
            tc.tile_pool(name="pacc", bufs=1, space="PSUM") as pacc,
            tc.tile_pool(name="plg", bufs=4, space="PSUM") as plg,
        ):
            # constants / one-time loads
            gw_sb = cpool.tile([P, NDT * E], fp32)
            nc.sync.dma_start(gw_sb[:], gw_in.ap())
            ident = cpool.tile([P, P], fp32)
            make_identity(nc, ident[:])
            # warm the ACT sigmoid table early (overlaps the input stream)
            scratch = cpool.tile([P, 1], fp32)
            nc.gpsimd.memset(scratch[:], 0.0)
            nc.scalar.activation(scratch[:], scratch[:], Sig)

            # merged output accumulator: weights bits then indices
            acc_all = opool.tile([P, NSEG * K * 2], mybir.dt.uint32)
            wv = acc_all[:, 0:NSEG * K].bitcast(fp32).rearrange(
                "p (s k) -> p s k", k=K)

            xT_v = xT_in.ap().rearrange("(dt p) t -> p dt t", p=P)

            # ---- phase 1: issue EVERY input sub-DMA before anything that
            # could block the sync queue ----
            subs = {}
            for si, (t0, d0, nd) in enumerate(PACKED_SUBS):
                xt = xpool.tile([P, PK_W], fp32, tag=f"x{t0}_{d0}",
                                name=f"xt_{t0}_{d0}")
                nc.sync.dma_start(xt[:],
                                  xpk_in.ap()[:, si * PK_W:(si + 1) * PK_W])
                subs[(t0, d0, nd)] = xt
            for (d0, nd) in C2_SPLITS:
                xt = xpool.tile([P, nd * 512], fp32,
                                tag=f"x1536_{d0}", name=f"xt_1536_{d0}")
                nc.sync.dma_start(
                    xt[:].rearrange("p (dt t) -> p dt t", dt=nd),
                    xT_v[:, d0:d0 + nd, 512:1024],
                )
                subs[(1536, d0, nd)] = xt

            def find_src(t0, ntok, dt):
                for (ct0, d0, nd), xt in subs.items():
                    if ct0 == t0 and d0 <= dt < d0 + nd:
                        return xt, (dt - d0) * ntok
                raise AssertionError

            # ---- phase 2: per-chunk matmuls + epilogue ----
            for ci, (t0, ntok) in enumerate(CHUNKS):
                half = ntok // 2
                nblk = ntok // P
                s0 = t0 // P

                # col-packed fp32 matmul pair per d-tile: group A (tokens
                # [0, half)) -> PSUM rows 0:64, group B -> rows 64:128
                acc = pacc.tile([P, half], fp32, tag=f"acc{ci}",
                                name=f"acc{ci}")
                for dt in range(NDT):
                    src, base = find_src(t0, ntok, dt)
                    gsl = gw_sb[:, dt * E:(dt + 1) * E]
                    mmargs = dict(start=(dt == 0), stop=(dt == NDT - 1))
                    nc.tensor.matmul(acc[0:64, :], gsl,
                                     src[:, base:base + half],
                                     tile_position=(0, 0), **mmargs)
                    nc.tensor.matmul(acc[64:128, :], gsl,
                                     src[:, base + half:base + ntok],
                                     tile_position=(0, 64), **mmargs)

                # quadrant copies into token-aligned [128, ntok] layout
                lt = epool.tile([P, ntok], fp32, tag=f"lt{ci}",
                                name=f"lt{ci}")
                nc.vector.tensor_copy(lt[0:64, 0:half], acc[0:64, :])
                nc.vector.tensor_copy(lt[64:128, half:ntok], acc[64:128, :])

                # back-transpose per 128-token block into PSUM in 2-block
                # batches; top-8 reads PSUM directly
                mx = opool.tile([P, nblk * 8], fp32, tag=f"mx{ci}",
                                name=f"mx{ci}")
                mi = opool.tile([P, nblk * 8], mybir.dt.uint32,
                                tag=f"mi{ci}", name=f"mi{ci}")
                mi3 = mi[:].rearrange("p (s k) -> p s k", k=8)
                mx3 = mx[:].rearrange("p (s k) -> p s k", k=8)

                lgs = {}
                for b0 in range(0, nblk, 2):
                    lg = plg.tile([P, 256], fp32, tag="lg",
                                  name=f"lg{ci}_{b0}")
                    for j in (b0, b0 + 1):
                        lgs[j] = (lg, (j - b0) * P)
                        nc.tensor.transpose(
                            lg[:, (j - b0) * P:(j - b0 + 1) * P],
                            lt[:, j * P:(j + 1) * P], ident[:],
                        )
                    # max8 first: the softmax chain depends only on these
                    for j in (b0, b0 + 1):
                        cb = 0 if j < nblk // 2 else 64
                        lgt, lo = lgs[j]
                        nc.vector.max(out=mx[:, j * 8:(j + 1) * 8],
                                      in_=lgt[:, lo + cb:lo + cb + 64])

                # top-2 softmax == sigmoid of the score gap (both weights);
                # runs on ACT while the DVE does the index finds below
                delta = epool.tile([P, nblk], fp32, tag=f"dl{ci}",
                                   name=f"dl{ci}")
                nc.vector.tensor_tensor(delta[:], mx3[:, :, 1], mx3[:, :, 0],
                                        op=mybir.AluOpType.subtract)
                nc.scalar.activation(wv[:, s0:s0 + nblk, 1], delta[:], Sig)
                nc.scalar.activation(wv[:, s0:s0 + nblk, 0], delta[:], Sig,
                                     scale=-1.0)

                for j in range(nblk):
                    cb = 0 if j < nblk // 2 else 64
                    lgt, lo = lgs[j]
                    nc.vector.max_index(
                        mi[:, j * 8:(j + 1) * 8],
                        mx[:, j * 8:(j + 1) * 8],
                        lgt[:, lo + cb:lo + cb + 64],
                    )
                nc.vector.tensor_copy(
                    acc_all[:, NSEG * K + s0 * K: NSEG * K + (s0 + nblk) * K]
                    .rearrange("p (s k) -> p s k", k=K),
                    mi3[:, :, 0:K])

            # ---- phase 3: one merged output DMA ----
            nc.sync.dma_start(o_out.ap(), acc_all[:])

    nc.compile()
    return nc


def _get_compiled():
    global _compiled
    if _compiled is None:
        _compiled = _build()
    return _compiled


def kernel(x, gate_w):
    from concourse.bass_utils import run_bass_kernel_spmd

    x = np.ascontiguousarray(np.asarray(x, dtype=np.float32))
    gate_w = np.ascontiguousarray(np.asarray(gate_w, dtype=np.float32))
    assert x.shape == (B, T, D) and gate_w.shape == (E, D)

    nc = _get_compiled()

    x_flat = x.reshape(B * T, D)
    # gate_w.T laid out [128, 16*64]: (p, dt*64+e) = gate_w[e, dt*128+p]
    gwl = np.ascontiguousarray(
        gate_w.T.reshape(NDT, P, E).transpose(1, 0, 2).reshape(P, NDT * E)
    )

    from concurrent.futures import ThreadPoolExecutor

    def shard(c):
        sl = x_flat[c * TOK_PER_CORE:(c + 1) * TOK_PER_CORE]
        xT = np.ascontiguousarray(sl.T)            # [D, TOK_PER_CORE]
        xs = xT.reshape(NDT, P, TOK_PER_CORE)      # [dt, p, t]
        blocks = []
        for (t0, d0, nd) in PACKED_SUBS:
            ntok = dict(CHUNKS)[t0]
            blk = xs[d0:d0 + nd, :, t0:t0 + ntok]  # [nd, p, t]
            blocks.append(np.ascontiguousarray(
                blk.transpose(1, 0, 2)).reshape(P, nd * ntok))
        xpk = np.concatenate(blocks, axis=1)       # [P, 6*4096]
        xT2 = np.ascontiguousarray(xT[:, 1024:2048])
        return xpk, xT2

    with ThreadPoolExecutor(max_workers=N_CORES) as ex:
        shards = list(ex.map(shard, range(N_CORES)))

    in_maps = [{"xpk": shards[c][0], "xT2": shards[c][1], "gwl": gwl}
               for c in range(N_CORES)]
    res = run_bass_kernel_spmd(nc, in_maps, list(range(N_CORES)))

    # device buffer is [P, 2*NSEG*K] u32: first half f32 weight bits,
    # second half indices; token = s*128 + p
    def unperm(buf):
        return buf.reshape(P, NSEG, K).transpose(1, 0, 2).reshape(
            TOK_PER_CORE, K)

    ws, idxs = [], []
    for c in range(N_CORES):
        o = res.results[c]["o"]
        ws.append(unperm(o[:, :NSEG * K].view(np.float32)))
        idxs.append(unperm(o[:, NSEG * K:]))
    weights = np.concatenate(ws, axis=0).reshape(B, T, K).astype(np.float32)
    indices = np.concatenate(idxs, axis=0).reshape(B, T, K).astype(np.int32)
    return weights, indices
